# revision 33
# baseline (speedup 1.0000x reference)
"""BNNLinear sampling kernel for Trainium2, data-parallel over 8 NeuronCores.

Computes h[m,c] = sum_r x_ext[m,r] * (mu[c,r] + sqrt(var[c,r]) * E[m,c,r])
with x_ext = concat([x, ones], axis=1), for
  x  [256, 512] f32, mu/var [512, 513] f32, E [256, 512, 513] f32.

Strategy (memory-bound; E dominates HBM traffic and must stream once):
 - Shard the sample axis m across the 8 cores (32 samples each).
 - Host-side prep is layout + dtype only: inputs are cast to bf16 (the
   quantization error on the final h is ~2.5e-3 relative, well inside the
   2e-2 gate) and E is per-sample transposed to [r, c] blocked as
   [pair, p, j, k, c] (r = 128k + p, m = 2*pair + j) so each DMA is one
   contiguous 1 MB transfer covering two samples. All arithmetic (sqrt,
   multiplies, reductions) is on-chip.
 - Ring split: the E stream issues on the SP HWDGE ring; all constant /
   auxiliary / output DMAs go on the Activation HWDGE ring so they never
   stall the stream's descriptor dispatch.
 - Per sample: one DVE tensor_tensor B = E_t * sqrt(var)_t ([128, 2048]
   bf16, 2x perf mode) against the shared s tile, then 5 bf16 PE matmuls:
   4 accumulate sum_r x*B over the r-chunks into a PSUM row, and a 5th
   adds the precomputed row hbs[m] = (x_t @ mu_t + mu_bias + sqrt(var_b) *
   E_bias)[m] via a ones[1,1] stationary against a partition-0 bias tile.
   The finished PSUM bank (4 samples) DMAs straight to the DRAM output.
 - The last 3 pairs stream in 4 r-chunk DMAs each (chunk-major layout) with
   per-chunk TT/matmul so the DVE work interleaves finely with the tail of
   the wire and the post-last-byte serial chain is short.
"""

import numpy as np
from contextlib import ExitStack

import ml_dtypes

import concourse.bacc as bacc
import concourse.mybir as mybir
import concourse.tile as tile
from concourse.bass_utils import run_bass_kernel_spmd

F32 = mybir.dt.float32
BF16 = mybir.dt.bfloat16
NP_BF16 = ml_dtypes.bfloat16

N_CORES = 8
M_TOTAL = 256
M_SH = M_TOTAL // N_CORES  # 32 samples per core
NPAIR = M_SH // 2          # 16 E-pairs per core; the last NCHUNKED stream chunked
NCHUNKED = 1
C = 512
R_IN = 512                 # r chunks: 4 x 128
KCH = 4

_COMPILED = None


def _build_program(repeat=1):
    nc = bacc.Bacc("TRN2", target_bir_lowering=False, debug=False)

    et_d = nc.dram_tensor(
        "et", [NPAIR - NCHUNKED, 128, 2, KCH, C], BF16, kind="ExternalInput"
    ).ap()
    etl_d = nc.dram_tensor(
        "etl", [NCHUNKED, KCH, 128, 2, C], BF16, kind="ExternalInput"
    ).ap()
    eb_d = nc.dram_tensor("eb", [M_SH, C], BF16, kind="ExternalInput").ap()
    xt_d = nc.dram_tensor("xt", [128, KCH, M_SH], BF16, kind="ExternalInput").ap()
    mu_d = nc.dram_tensor("mu_t", [128, KCH, C], BF16, kind="ExternalInput").ap()
    mub_d = nc.dram_tensor("mu_b", [1, C], BF16, kind="ExternalInput").ap()
    var_d = nc.dram_tensor("var_t", [128, KCH, C], BF16, kind="ExternalInput").ap()
    varb_d = nc.dram_tensor("var_b", [1, C], BF16, kind="ExternalInput").ap()
    out_d = nc.dram_tensor("out", [M_SH, C], F32, kind="ExternalOutput").ap()

    with tile.TileContext(nc) as tc, ExitStack() as ctx:
        const = ctx.enter_context(tc.tile_pool(name="const", bufs=1))
        work = ctx.enter_context(tc.tile_pool(name="work", bufs=8))
        bpool = ctx.enter_context(tc.tile_pool(name="bpool", bufs=6))
        cpool = ctx.enter_context(tc.tile_pool(name="cpool", bufs=6))
        opool = ctx.enter_context(tc.tile_pool(name="opool", bufs=3))
        psum = ctx.enter_context(tc.tile_pool(name="psum", bufs=4, space="PSUM"))
        psum1 = ctx.enter_context(tc.tile_pool(name="psum1", bufs=1, space="PSUM"))

        # ---- constants ----
        # A dummy sqrt fires first so the ACT function-table load (~1.3 us)
        # overlaps the var DMAs instead of delaying the first real sqrt.
        dummy = const.tile([1, 16], BF16)
        nc.vector.memset(dummy[:], 1.0)
        dummy2 = const.tile([1, 16], BF16)
        nc.scalar.sqrt(dummy2[:], dummy[:])

        # var rides the SP ring AHEAD of the E stream (program order = ring
        # order) so sqrt(var) is ready ~5 us in and the first E TT can fire
        # as soon as pair 0 lands.
        var_sb = const.tile([128, KCH, C], BF16)
        nc.sync.dma_start(var_sb[:], var_d)
        s_sb = const.tile([128, KCH, C], BF16)
        nc.scalar.sqrt(s_sb[:], var_sb[:])

        xt_sb = const.tile([128, KCH, M_SH], BF16)
        nc.scalar.dma_start(xt_sb[:], xt_d)
        varb_sb = const.tile([1, C], BF16)
        nc.scalar.dma_start(varb_sb[:], varb_d)
        mu_sb = const.tile([128, KCH, C], BF16)
        nc.scalar.dma_start(mu_sb[:], mu_d)
        mub_sb = const.tile([1, C], BF16)
        nc.scalar.dma_start(mub_sb[:], mub_d)
        eb_sb = const.tile([M_SH, C], BF16)
        nc.scalar.dma_start(eb_sb[:], eb_d)

        sb_sb = const.tile([1, C], BF16)
        nc.scalar.sqrt(sb_sb[:], varb_sb[:])

        ones32 = const.tile([1, M_SH], BF16)
        nc.vector.memset(ones32[:], 1.0)
        ones1 = const.tile([1, 1], BF16)
        nc.vector.memset(ones1[:], 1.0)

        # broadcast sqrt(var) bias row to 32 partitions via PE outer product
        ps_b = psum1.tile([M_SH, C], F32)
        nc.tensor.matmul(ps_b[:], lhsT=ones32[:], rhs=sb_sb[:], start=True, stop=True)
        sbb_sb = const.tile([M_SH, C], BF16)
        nc.scalar.copy(sbb_sb[:], ps_b[:])

        # h1 = x_t @ mu_t + mu bias row  -> [32, 512] psum, rows = samples
        h1_ps = psum1.tile([M_SH, C], F32)
        for k in range(KCH):
            nc.tensor.matmul(
                h1_ps[:], lhsT=xt_sb[:, k, :], rhs=mu_sb[:, k, :],
                start=(k == 0), stop=False,
            )
        nc.tensor.matmul(h1_ps[:], lhsT=ones32[:], rhs=mub_sb[:], start=False, stop=True)

        # hbs[m, c] = h1[m, c] + Eb[m, c] * sqrt(var)[c, 512]
        ebs_sb = const.tile([M_SH, C], BF16)
        nc.vector.tensor_tensor(
            out=ebs_sb[:], in0=eb_sb[:], in1=sbb_sb[:], op=mybir.AluOpType.mult
        )
        hbs_sb = const.tile([M_SH, C], F32)
        nc.vector.tensor_tensor(
            out=hbs_sb[:], in0=h1_ps[:], in1=ebs_sb[:], op=mybir.AluOpType.add
        )
        # re-layout hbs rows: sample m = 4b + g  ->  partition 32g, block b
        # (one DMA per block: strided-dst APs with >1 free dim mis-lower)
        n_blk = M_SH // 4  # 8
        hbs_blk = const.tile([128, n_blk, C], F32)
        for b in range(n_blk):
            nc.scalar.dma_start(hbs_blk[0:128:32, b, :], hbs_sb[4 * b : 4 * b + 4, :])

        def sample_mms(ps, m, bt_k, nk=KCH, k0=0):
            """PE matmuls accumulating sum_r x[m,r]*B[r,c] into a PSUM row.
            PE must depend only on the DVE TTs (which stay ahead of it) —
            any extra cross-engine wait stalls PE and drops its clock ramp
            out of the full-speed p-state."""
            g = m % 4
            for k in range(k0, k0 + nk):
                nc.tensor.matmul(
                    ps[32 * g : 32 * g + 1, :],
                    lhsT=xt_sb[:, k, m : m + 1],
                    rhs=bt_k[k - k0],
                    start=(k == 0),
                    stop=(k == KCH - 1),
                    tile_position=(0, 32 * g),
                )

        # ---- main loop over sample pairs ----
        ps = None
        for r_i, p in [(rr, pp) for rr in range(repeat) for pp in range(NPAIR)]:
            b = p // 2
            if p % 2 == 0:
                ps = psum.tile([128, C], F32, tag="ps")
            if p >= NPAIR - NCHUNKED:
                # tail pairs: stream per r-chunk (chunk-major layout) so the
                # DVE TT work interleaves finely with the end of the wire
                # and the post-last-byte serial chain is short
                for k in range(KCH):
                    ech = cpool.tile([128, 2, C], BF16, tag="ech")
                    nc.sync.dma_start(ech[:], etl_d[p - (NPAIR - NCHUNKED), k])
                    btc = cpool.tile([128, 2, C], BF16, tag="btc")
                    for j in range(2):
                        nc.vector.tensor_tensor(
                            out=btc[:, j, :], in0=ech[:, j, :], in1=s_sb[:, k, :],
                            op=mybir.AluOpType.mult,
                        )
                    for j in range(2):
                        sample_mms(ps, 2 * p + j, [btc[:, j, :]], nk=1, k0=k)
            else:
                e_t = work.tile([128, 2, KCH, C], BF16, tag="et")
                nc.sync.dma_start(e_t[:], et_d[p])
                for j in range(2):
                    m = 2 * p + j
                    bt = bpool.tile([128, KCH, C], BF16, tag="bt")
                    nc.vector.tensor_tensor(
                        out=bt[:], in0=e_t[:, j], in1=s_sb[:], op=mybir.AluOpType.mult
                    )
                    sample_mms(ps, m, [bt[:, k, :] for k in range(KCH)])
            if p % 2 == 1:
                # drain bank b (4 finished sample rows): DVE adds the hbs
                # bias block (DMA cannot read PSUM), then DMA the block out
                ob = opool.tile([128, C], F32, tag="ob")
                nc.vector.tensor_tensor(
                    out=ob[0:97, :], in0=ps[0:97, :], in1=hbs_blk[0:97, b, :],
                    op=mybir.AluOpType.add,
                )
                nc.scalar.dma_start(out_d[4 * b : 4 * b + 4, :], ob[0:128:32, :])

    nc.compile()
    return nc


def _prep_inputs(x, mu, var, E):
    x = np.asarray(x, dtype=np.float32)
    mu = np.asarray(mu, dtype=np.float32)
    var = np.asarray(var, dtype=np.float32)
    E = np.asarray(E, dtype=np.float32)

    # mu/var transposed-blocked: [p, k, c] with r = 128k + p (r < 512)
    def blk(t):
        tt = np.ascontiguousarray(t.T[:R_IN])          # [512, 512] (r, c)
        return np.ascontiguousarray(
            tt.reshape(KCH, 128, C).transpose(1, 0, 2)  # [128, 4, 512]
        ).astype(NP_BF16)

    mu_t = blk(mu)
    var_t = blk(var)
    mu_b = mu[:, R_IN].reshape(1, C).astype(NP_BF16)
    var_b = var[:, R_IN].reshape(1, C).astype(NP_BF16)

    # E per-sample transpose + block + pair: [pair, p, j, k, c], m = 2*pair+j
    et = np.ascontiguousarray(
        E.transpose(0, 2, 1)[:, :R_IN, :]              # [256, 512(r), 512(c)]
        .reshape(M_TOTAL, KCH, 128, C)
        .transpose(0, 2, 1, 3)                          # [256, 128, 4, 512]
    ).astype(NP_BF16)
    etp = np.ascontiguousarray(
        et.reshape(M_TOTAL // 2, 2, 128, KCH, C).transpose(0, 2, 1, 3, 4)
    )                                                   # [128pair, 128, 2, 4, 512]
    eb = np.ascontiguousarray(E[:, :, R_IN]).astype(NP_BF16)  # [256, 512]

    # x transposed-blocked per core: [p, k, m_local]
    in_maps = []
    for core in range(N_CORES):
        sl = slice(core * M_SH, (core + 1) * M_SH)
        p0 = core * NPAIR
        xs = x[sl]                                      # [32, 512]
        xt = np.ascontiguousarray(
            xs.T.reshape(KCH, 128, M_SH).transpose(1, 0, 2)  # [128, 4, 32]
        ).astype(NP_BF16)
        # tail pairs chunk-major: [q, k, p, j, c]
        etl = np.ascontiguousarray(
            etp[p0 + NPAIR - NCHUNKED : p0 + NPAIR].transpose(0, 3, 1, 2, 4)
        )                                               # [NCHUNKED, 4, 128, 2, 512]
        in_maps.append({
            "et": np.ascontiguousarray(etp[p0 : p0 + NPAIR - NCHUNKED]),
            "etl": etl,
            "eb": np.ascontiguousarray(eb[sl]),
            "xt": xt,
            "mu_t": mu_t,
            "var_t": var_t,
            "mu_b": mu_b,
            "var_b": var_b,
        })
    return in_maps


def kernel(x, mu, var, E, shape=None, _trace=False, **_ignored):
    global _COMPILED
    if _COMPILED is None:
        _COMPILED = _build_program()
    nc = _COMPILED
    in_maps = _prep_inputs(np.asarray(x), np.asarray(mu), np.asarray(var), np.asarray(E))
    res = run_bass_kernel_spmd(
        nc, in_maps, core_ids=list(range(N_CORES)), trace=_trace,
    )
    out = np.concatenate([res.results[i]["out"] for i in range(N_CORES)], axis=0)
    if _trace:
        kernel._last_results = res
    return out
